# revision 1
# baseline (speedup 1.0000x reference)
"""Siddon DRR kernel for 8 Trainium2 NeuronCores.

Ray-parallel: 40000 rays split 8 ways. Volume (flipped, bf16) on every core
as 256 x-slab ap_gather tables (2 generations of 128 slabs, one per SBUF
partition). Per (ray, x-slab) alpha window there are at most 3 y/z plane
crossings for this geometry, so each window is exactly 4 Siddon intervals:
boundaries [t0,x1,x2,x3,t1] from a 2+2 merge of per-axis crossing candidates
(closed-form counts of arithmetic alpha sequences). Widths/midpoints/voxel
indices computed densely on DVE/ACT, voxel fetch via GPSIMD ap_gather (d=2
bf16 pairs), weighted 4-slot segmented reduce on DVE, cross-partition sum
via PE ones-matmul, per-ray ||sdd|| scale, DMA out.
"""
import sys
sys.path.insert(0, "/opt/trn_rl_repo")
import contextlib
import numpy as np

import concourse.bass as bass
import concourse.bacc as bacc
import concourse.tile as tile
import concourse.mybir as mybir
from concourse.bass_utils import run_bass_kernel_spmd
from ml_dtypes import bfloat16

F32 = np.float32
AOP = mybir.AluOpType
ACTF = mybir.ActivationFunctionType

H = W = 200
NXV = NYV = NZV = 256
DELX = DELY = 1.5
EPS = 1e-8
NCORES = 8
NRAY = H * W
NPC = NRAY // NCORES          # rays per core (5000)
RC = 96                       # rays per chunk
NCH = 54                      # chunks per generation
NPAD = RC * NCH               # padded rays per core (5184)
NP = 15                       # packed params per ray
MAGIC = F32(12582912.0)       # 2^23 + 2^22: round-to-nearest for |x|<2^22
BIG = F32(3.0e38)
NE = 32768                    # bf16 pairs per slab table

_cache = {}


def _host_geometry(spacing, sdr, rotations, translations):
    """Per-ray launch parameters, float32 (matches reference numerics)."""
    sp = np.asarray(spacing, F32)
    sdrf = F32(np.asarray(sdr).reshape(-1)[0])
    rot = np.asarray(rotations, F32)
    tr = np.asarray(translations, F32)
    th, ph, ga = rot[0]
    ct, st = np.cos(th, dtype=F32), np.sin(th, dtype=F32)
    cp, spn = np.cos(ph, dtype=F32), np.sin(ph, dtype=F32)
    cg, sg = np.cos(ga, dtype=F32), np.sin(ga, dtype=F32)
    Rz = np.array([[ct, -st, 0], [st, ct, 0], [0, 0, 1]], F32)
    Ry = np.array([[cp, 0, spn], [0, 1, 0], [-spn, 0, cp]], F32)
    Rx = np.array([[1, 0, 0], [0, cg, -sg], [0, sg, cg]], F32)
    Rm = (Rz @ Ry @ Rx).astype(F32)
    source3 = (sdrf * Rm[:, 0]).astype(F32)
    center3 = (-source3).astype(F32)
    basis = np.stack([Rm[:, 1], Rm[:, 2]]).astype(F32)
    source3 = source3 + tr[0]
    center3 = center3 + tr[0]
    t = (np.arange(-(H // 2), H // 2, dtype=F32) + F32(1.0)) * F32(DELX)
    s = (np.arange(-(W // 2), W // 2, dtype=F32) + F32(1.0)) * F32(DELY)
    coefs = np.stack(np.meshgrid(t, s, indexing="ij"), -1).reshape(-1, 2).astype(F32)
    target = (coefs @ basis + center3).astype(F32)
    sdd = (target - source3 + F32(EPS)).astype(F32)

    a0 = ((F32(0.0) - source3) / sdd).astype(F32)
    extent = (np.array([NXV, NYV, NZV], F32) * sp).astype(F32)
    a1 = ((extent - source3) / sdd).astype(F32)
    amin = np.minimum(a0, a1).max(-1).astype(F32)
    amax = np.maximum(a0, a1).min(-1).astype(F32)
    rnorm = np.sqrt((sdd * sdd).sum(-1)).astype(F32)

    hx, hy, hz = sp
    sx, sy, sz = source3
    p = np.zeros((NP, NRAY), F32)
    p[0] = sdd[:, 0]; p[1] = sdd[:, 1]; p[2] = sdd[:, 2]
    p[3] = amin; p[4] = amax
    Dy = np.where(sdd[:, 1] > 0, F32(1.0), F32(-1.0)).astype(F32)
    Dz = np.where(sdd[:, 2] > 0, F32(1.0), F32(-1.0)).astype(F32)
    p[5] = sdd[:, 1] * Dy / hy                       # alpha_y
    p[6] = sy * Dy / hy                              # beta_y
    p[7] = np.where(Dy > 0, F32(1.0), F32(257.0))    # offy
    p[8] = Dy
    p[9] = np.where(Dy > 0, F32(0.0), F32(256.0))    # m0y
    p[10] = sdd[:, 2] * Dz / hz
    p[11] = sz * Dz / hz
    p[12] = np.where(Dz > 0, F32(1.0), F32(257.0))
    p[13] = Dz
    p[14] = np.where(Dz > 0, F32(0.0), F32(256.0))
    scal = dict(hx=F32(hx), hy=F32(hy), hz=F32(hz),
                sx=F32(sx), sy=F32(sy), sz=F32(sz))
    return p, rnorm, scal


def _build(scal):
    hx, hy, hz = scal["hx"], scal["hy"], scal["hz"]
    sx, sy, sz = scal["sx"], scal["sy"], scal["sz"]
    inv_hy, inv_hz = 1.0 / hy, 1.0 / hz

    nc = bacc.Bacc("TRN2", target_bir_lowering=False, debug=False,
                   num_devices=1)
    vol_in = nc.dram_tensor("vol", [2, 128, NYV * NZV],
                            mybir.dt.bfloat16, kind="ExternalInput")
    par_in = nc.dram_tensor("par", [NCH, 128, NP * RC], mybir.dt.float32,
                            kind="ExternalInput")
    rno_in = nc.dram_tensor("rno", [1, NPAD], mybir.dt.float32,
                            kind="ExternalInput")
    msk_in = nc.dram_tensor("msk", [128, 16], mybir.dt.float32,
                            kind="ExternalInput")
    drr_out = nc.dram_tensor("drr", [1, NPAD], mybir.dt.float32,
                             kind="ExternalOutput")
    s_hbm = nc.dram_tensor("s_part", [2, 128, NPAD], mybir.dt.float32)

    with tile.TileContext(nc) as tc:
        with tc.tile_pool(name="pool", bufs=1) as pool, \
             tc.tile_pool(name="ppsum", bufs=2, space="PSUM") as ppsum:
            table = pool.tile([128, NYV * NZV], mybir.dt.bfloat16,
                              tag="table")
            jci = pool.tile([128, 1], mybir.dt.int32, tag="jci")
            jc0 = pool.tile([128, 1], mybir.dt.float32, tag="jc0")
            nc.gpsimd.iota(jci[:], pattern=[[0, 1]], base=0,
                           channel_multiplier=1)
            nc.vector.tensor_copy(jc0[:], jci[:])   # partition index p
            msk = pool.tile([128, 16], mybir.dt.float32, tag="msk")
            nc.sync.dma_start(msk[:], msk_in[:])

            def T(tag, w=1, dt=mybir.dt.float32):
                return pool.tile([128, w * RC], dt, tag=tag, name=tag)

            for g in range(2):
                nc.gpsimd.dma_start(table[:], vol_in[g])
                # per-partition window numerators: j = p + 128g
                njc0 = pool.tile([128, 1], mybir.dt.float32, tag="njc0")
                njc1 = pool.tile([128, 1], mybir.dt.float32, tag="njc1")
                # njc0 = j*hx - sx ; njc1 = (j+1)*hx - sx
                nc.vector.tensor_scalar(njc0[:], jc0[:], float(hx),
                                        float(128.0 * g * hx - sx),
                                        AOP.mult, AOP.add)
                nc.vector.tensor_scalar(njc1[:], jc0[:], float(hx),
                                        float(128.0 * g * hx - sx + hx),
                                        AOP.mult, AOP.add)
                for ch in range(NCH):
                    pt = pool.tile([128, NP * RC], mybir.dt.float32, tag="pt")
                    nc.sync.dma_start(pt[:], par_in[ch])
                    P = lambda i: pt[:, i * RC:(i + 1) * RC]
                    sddy, sddz = P(1), P(2)
                    amin, amax = P(3), P(4)
                    ay_, by_, offy, Dy, m0y = P(5), P(6), P(7), P(8), P(9)
                    az_, bz_, offz, Dz, m0z = P(10), P(11), P(12), P(13), P(14)

                    rsx, rsy, rsz = T("rsx"), T("rsy"), T("rsz")
                    nc.vector.reciprocal(rsx[:], P(0))
                    nc.vector.reciprocal(rsy[:], sddy)
                    nc.vector.reciprocal(rsz[:], sddz)

                    bb = pool.tile([128, 5 * RC], mybir.dt.float32, tag="bb")
                    t0 = bb[:, 0 * RC:1 * RC]
                    t1 = bb[:, 4 * RC:5 * RC]
                    axj, axj1 = T("axj"), T("axj1")
                    # axj = (j*hx-sx) * (1/sddx): per-partition scalar * tensor
                    nc.vector.tensor_scalar(axj[:], rsx[:], njc0[:, 0:1], None,
                                            AOP.mult)
                    nc.vector.tensor_scalar(axj1[:], rsx[:], njc1[:, 0:1],
                                            None, AOP.mult)
                    nc.vector.scalar_tensor_tensor(t0, axj[:], 1.0,
                                                   axj1[:], AOP.bypass,
                                                   AOP.min)
                    nc.vector.scalar_tensor_tensor(t0, t0, 1.0, amin,
                                                   AOP.bypass, AOP.max)
                    nc.vector.scalar_tensor_tensor(t1, axj[:], 1.0,
                                                   axj1[:], AOP.bypass,
                                                   AOP.max)
                    nc.vector.scalar_tensor_tensor(t1, t1, 1.0, amax,
                                                   AOP.bypass, AOP.min)

                    def counts(tt, al, be, off, outtag):
                        """C(t) = clamp(floor(t*al+be) + off, 0, 257)"""
                        gv, rv, fxs = T("cgv"), T("crv"), T("cfx")
                        nc.vector.scalar_tensor_tensor(gv[:], tt, 1.0,
                                                       al, AOP.bypass,
                                                       AOP.mult)
                        nc.vector.scalar_tensor_tensor(gv[:], gv[:], 1.0,
                                                       be, AOP.bypass, AOP.add)
                        nc.vector.tensor_scalar(rv[:], gv[:], float(MAGIC), float(MAGIC),
                                                AOP.add, AOP.subtract)
                        nc.vector.scalar_tensor_tensor(fxs[:], rv[:], 1.0,
                                                       gv[:], AOP.bypass,
                                                       AOP.is_gt)
                        out = T(outtag)
                        nc.vector.scalar_tensor_tensor(out[:], rv[:], 1.0,
                                                       fxs[:], AOP.bypass,
                                                       AOP.subtract)
                        nc.vector.scalar_tensor_tensor(out[:], out[:],
                                                       1.0, off,
                                                       AOP.bypass, AOP.add)
                        nc.vector.tensor_scalar(out[:], out[:], 0.0,
                                                257.0, AOP.max, AOP.min)
                        return out

                    def axis_events(al, be, off, Dv, m0, rs, hh, ss, tg):
                        C0 = counts(t0, al, be, off, "C0")
                        C1 = counts(t1, al, be, off, "C1")
                        cnt, m1, m2 = T("cnt"), T("m1"), T("m2")
                        nc.vector.scalar_tensor_tensor(cnt[:], C1[:], 1.0,
                                                       C0[:], AOP.bypass,
                                                       AOP.subtract)
                        nc.vector.scalar_tensor_tensor(m1[:], C0[:], 1.0,
                                                       Dv, AOP.bypass,
                                                       AOP.mult)
                        nc.vector.scalar_tensor_tensor(m1[:], m1[:], 1.0,
                                                       m0, AOP.bypass, AOP.add)
                        nc.vector.scalar_tensor_tensor(m2[:], m1[:], 1.0,
                                                       Dv, AOP.bypass, AOP.add)
                        e1, e2 = T(tg + "e1"), T(tg + "e2")
                        gg = T("gg", 1, mybir.dt.uint8)
                        for (mm, ee, thr) in ((m1, e1, 0.5),
                                              (m2, e2, 1.5)):
                            nc.vector.tensor_scalar(ee[:], mm[:], float(hh), float(-ss),
                                                    AOP.mult, AOP.add)
                            nc.vector.scalar_tensor_tensor(ee[:], ee[:],
                                                           1.0, rs[:],
                                                           AOP.bypass,
                                                           AOP.mult)
                            # invalid event -> amax sentinel (loses every min,
                            # clips to t1 => zero-width slot). Exact overwrite.
                            nc.vector.tensor_scalar(gg[:], cnt[:], thr, None,
                                                    AOP.is_le)
                            nc.vector.copy_predicated(ee[:], gg[:], amax)
                        return e1, e2

                    e1, e2 = axis_events(ay_, by_, offy, Dy, m0y, rsy, hy, sy,
                                         "y")
                    f1, f2 = axis_events(az_, bz_, offz, Dz, m0z, rsz, hz, sz,
                                         "z")

                    x1 = bb[:, 1 * RC:2 * RC]
                    x2 = bb[:, 2 * RC:3 * RC]
                    x3 = bb[:, 3 * RC:4 * RC]
                    hilo, lohi = T("axj"), T("axj1")
                    nc.vector.scalar_tensor_tensor(x1, e1[:], 1.0, f1[:],
                                                   AOP.bypass, AOP.min)
                    nc.vector.scalar_tensor_tensor(hilo[:], e1[:], 1.0,
                                                   f1[:], AOP.bypass, AOP.max)
                    nc.vector.scalar_tensor_tensor(lohi[:], e2[:], 1.0,
                                                   f2[:], AOP.bypass, AOP.min)
                    nc.vector.scalar_tensor_tensor(x2, hilo[:], 1.0,
                                                   lohi[:], AOP.bypass,
                                                   AOP.min)
                    nc.vector.scalar_tensor_tensor(x3, hilo[:], 1.0,
                                                   lohi[:], AOP.bypass,
                                                   AOP.max)
                    for xx in (x1, x2, x3):
                        nc.vector.scalar_tensor_tensor(xx, xx, 1.0, t0,
                                                       AOP.bypass, AOP.max)
                        nc.vector.scalar_tensor_tensor(xx, xx, 1.0, t1,
                                                       AOP.bypass, AOP.min)

                    wgt = T("wgt", 4)
                    mid = T("mid", 4)
                    bL = bb[:, 0:4 * RC]
                    bR = bb[:, RC:5 * RC]
                    nc.vector.scalar_tensor_tensor(wgt[:], bR, 1.0, bL,
                                                   AOP.bypass, AOP.subtract)
                    nc.vector.tensor_scalar(wgt[:], wgt[:], 0.0, None,
                                            AOP.max)
                    nc.vector.scalar_tensor_tensor(mid[:], bL, 1.0, bR,
                                                   AOP.bypass, AOP.add)
                    nc.vector.tensor_scalar(mid[:], mid[:], 0.5, None,
                                            AOP.mult)

                    mid3 = mid[:].rearrange("p (four r) -> p four r", four=4)

                    def slot_floor_idx(sdd_p, inv_h, ss, outtag):
                        """floor((ss + mid*sdd)/h) clip [0,255] on [128,4RC]"""
                        pos, fxs = T("spos", 4), T("sfx", 4)
                        pos3 = pos[:].rearrange("p (four r) -> p four r",
                                                four=4)
                        sdd_b = sdd_p.rearrange(
                            "p (one r) -> p one r", one=1).broadcast_to(
                            (128, 4, RC))
                        nc.vector.scalar_tensor_tensor(pos3, mid3, 1.0,
                                                       sdd_b, AOP.bypass,
                                                       AOP.mult)
                        nc.vector.tensor_scalar(pos[:], pos[:], float(ss), float(inv_h),
                                                AOP.add, AOP.mult)
                        rv = T(outtag, 4)
                        nc.vector.tensor_scalar(rv[:], pos[:], float(MAGIC), float(MAGIC),
                                                AOP.add, AOP.subtract)
                        nc.vector.scalar_tensor_tensor(fxs[:], rv[:], 1.0,
                                                       pos[:], AOP.bypass,
                                                       AOP.is_gt)
                        nc.vector.scalar_tensor_tensor(rv[:], rv[:], 1.0,
                                                       fxs[:], AOP.bypass,
                                                       AOP.subtract)
                        nc.vector.tensor_scalar(rv[:], rv[:], 0.0,
                                                255.0, AOP.max, AOP.min)
                        return rv

                    iy = slot_floor_idx(sddy, inv_hy, sy, "iy")
                    iz = slot_floor_idx(sddz, inv_hz, sz, "iz")

                    izh, odd = T("izh", 4), T("bb", 4)
                    zh = T("sfx", 4)
                    nc.vector.tensor_scalar(zh[:], iz[:], 0.5, None, AOP.mult)
                    nc.vector.tensor_scalar(izh[:], zh[:], float(MAGIC),
                                            float(MAGIC), AOP.add,
                                            AOP.subtract)
                    zfx = T("spos", 4)
                    nc.vector.scalar_tensor_tensor(zfx[:], izh[:], 1.0, zh[:],
                                                   AOP.bypass, AOP.is_gt)
                    nc.vector.scalar_tensor_tensor(izh[:], izh[:], 1.0,
                                                   zfx[:], AOP.bypass,
                                                   AOP.subtract)
                    nc.vector.scalar_tensor_tensor(odd[:], izh[:], -2.0,
                                                   iz[:], AOP.mult, AOP.add)
                    pairf = T("mid", 4)
                    nc.vector.scalar_tensor_tensor(pairf[:], iy[:], 128.0,
                                                   izh[:], AOP.mult, AOP.add)
                    idx16 = pool.tile([128, 4 * RC], mybir.dt.int16,
                                      tag="idx16")
                    nc.vector.tensor_copy(idx16[:], pairf[:])

                    gt = pool.tile([128, 4 * RC, 16, 2], mybir.dt.bfloat16,
                                   tag="gt")
                    nc.gpsimd.ap_gather(
                        out_ap=gt[:].rearrange("p a b c -> p (a b c)"),
                        in_ap=table[:], idxs_ap=idx16[:],
                        channels=128, num_elems=NE, d=2, num_idxs=16 * 4 * RC)

                    dvW = pool.tile([128, 4 * RC * 16], mybir.dt.float32,
                                    tag="dvW", name="dvW")
                    wmW = pool.tile([128, 4 * RC * 16], mybir.dt.float32,
                                    tag="gt", name="wmW")
                    dvW3 = dvW[:].rearrange("p (f q) -> p f q", q=16)
                    wmW3 = wmW[:].rearrange("p (f q) -> p f q", q=16)
                    glo = gt[:, :, :, 0]
                    ghi = gt[:, :, :, 1]
                    oddB = odd[:].rearrange("p (f one) -> p f one",
                                            one=1).broadcast_to((128, 4 * RC, 16))
                    wgtB = wgt[:].rearrange("p (f one) -> p f one",
                                            one=1).broadcast_to((128, 4 * RC, 16))
                    mskB = msk[:].rearrange("p (one q) -> p one q",
                                            one=1).broadcast_to((128, 4 * RC, 16))
                    nc.vector.scalar_tensor_tensor(dvW3, ghi, 1.0, glo,
                                                   AOP.bypass, AOP.subtract)
                    nc.vector.scalar_tensor_tensor(dvW3, dvW3, 1.0, oddB,
                                                   AOP.bypass, AOP.mult)
                    nc.vector.scalar_tensor_tensor(dvW3, dvW3, 1.0, glo,
                                                   AOP.bypass, AOP.add)
                    nc.vector.scalar_tensor_tensor(wmW3, wgtB, 1.0, mskB,
                                                   AOP.bypass, AOP.mult)
                    nc.vector.scalar_tensor_tensor(dvW3, dvW3, 1.0, wmW3,
                                                   AOP.bypass, AOP.mult)
                    t4 = T("spos", 4)
                    nc.vector.tensor_reduce(t4[:], dvW3,
                                            axis=mybir.AxisListType.X,
                                            op=AOP.add)
                    sch = T("sch")
                    nc.vector.tensor_reduce(
                        sch[:],
                        t4[:].rearrange("p (four r) -> p r four", four=4),
                        axis=mybir.AxisListType.X, op=AOP.add)
                    nc.sync.dma_start(s_hbm[g, :, ch * RC:(ch + 1) * RC],
                                      sch[:])

            # DRR[r] = rnorm[r] * sum_p (s0[p,r] + s1[p,r])
            ones = pool.tile([128, 1], mybir.dt.float32, tag="ones")
            nc.vector.memset(ones[:], 1.0)
            CH2 = 432
            for nb in range(NPAD // CH2):
                sl = slice(nb * CH2, (nb + 1) * CH2)
                s0 = pool.tile([128, CH2], mybir.dt.float32, tag="gt", name="s0")
                s1 = pool.tile([128, CH2], mybir.dt.float32, tag="dvW", name="s1")
                nc.sync.dma_start(s0[:], s_hbm[0, :, sl])
                nc.sync.dma_start(s1[:], s_hbm[1, :, sl])
                nc.vector.scalar_tensor_tensor(s0[:], s0[:], 1.0, s1[:],
                                               AOP.bypass, AOP.add)
                ps = ppsum.tile([1, CH2], mybir.dt.float32)
                nc.tensor.matmul(ps[:], ones[:], s0[:], start=True, stop=True)
                rnc = pool.tile([1, CH2], mybir.dt.float32, tag="pt", name="rnc")
                nc.sync.dma_start(rnc[:], rno_in[:, sl])
                orow = pool.tile([1, CH2], mybir.dt.float32, tag="bb", name="orow")
                nc.vector.scalar_tensor_tensor(orow[:], ps[:], 1.0,
                                               rnc[:], AOP.bypass, AOP.mult)
                nc.sync.dma_start(drr_out[:, sl], orow[:])

    nc.finalize()
    return nc


def kernel(volume, spacing, sdr, rotations, translations):
    vol = np.asarray(volume, F32)
    par, rnorm, scal = _host_geometry(spacing, sdr, rotations, translations)
    if "nc" not in _cache:
        _cache["nc"] = _build(scal)
    nc = _cache["nc"]

    volf = vol[::-1].astype(bfloat16)                    # reference flip
    vol_dev = np.ascontiguousarray(volf.reshape(2, 128, NYV * NZV))

    in_maps = []
    for c in range(NCORES):
        lo = c * NPC
        pc = np.zeros((NP, NPAD), F32)
        pc[:, :NPC] = par[:, lo:lo + NPC]
        pc[:, NPC:] = pc[:, :1]                          # pad with ray 0
        pch = np.ascontiguousarray(
            pc.reshape(NP, NCH, RC).transpose(1, 0, 2)).reshape(NCH, 1,
                                                                NP * RC)
        pch = np.ascontiguousarray(np.broadcast_to(pch, (NCH, 128, NP * RC)))
        rn = np.zeros((1, NPAD), F32)
        rn[0, :NPC] = rnorm[lo:lo + NPC]
        mk = (np.arange(128)[:, None] % 16 ==
              np.arange(16)[None, :]).astype(F32)
        in_maps.append({"vol": vol_dev, "par": pch, "rno": rn, "msk": mk})

    _cache["in_maps"] = in_maps
    res = run_bass_kernel_spmd(nc, in_maps, list(range(NCORES)))
    drr = np.concatenate([res.results[c]["drr"][0, :NPC]
                          for c in range(NCORES)])
    return drr.reshape(1, H, W).astype(F32)



# revision 9
# speedup vs baseline: 5.4434x; 5.4434x over previous
"""Siddon DRR kernel v2 for 8 Trainium2 NeuronCores.

Instruction-count-minimized redesign. Measured backend cost model:
~28us fixed per DVE op + ~3.5ns/elem, ap_gather ~45us + 34ns/idx,
DMA ~50us, no cross-engine overlap. So: few, wide ops.

Structure (per core): volume (flipped, bf16) as 256 x-slab tables in 2
generations of 128 partitions. Rays chunked RC=176 x 29 chunks. Per
(ray, slab) window [t0,t1]: at most 2 y- and 2 z-plane crossings; the 3
smallest events + window ends give 4 Siddon sub-intervals. Events from
floor(g(t0)) via round-to-nearest MAGIC with -0.5 host-folded offsets
(exact-integer edge cases accepted). Voxel fetch: one ap_gather of 4
slots x 16-shared-idx pairs; select via mask+interleaved-weight mults
and a fused (q,c)-axis reduce. Cross-partition sum via PE ones-matmul.
"""
import sys
sys.path.insert(0, "/opt/trn_rl_repo")
import numpy as np

import concourse.bass as bass
import concourse.bacc as bacc
import concourse.tile as tile
import concourse.mybir as mybir
from concourse.bass_utils import run_bass_kernel_spmd
from ml_dtypes import bfloat16

F32 = np.float32
AOP = mybir.AluOpType
BF16 = mybir.dt.bfloat16
MF32 = mybir.dt.float32

H = W = 200
NXV = NYV = NZV = 256
DELX = DELY = 1.5
EPS = 1e-8
NCORES = 8
NRAY = H * W
NPC = NRAY // NCORES          # rays per core (5000)
RC = 176                      # rays per chunk
NCH = 29                      # chunks per generation
NPAD = RC * NCH               # padded rays per core (5104)
NP2 = 23                      # packed params per ray
MAGIC = F32(12582912.0)       # 2^23 + 2^22: rne for |x|<2^22
NE = 32768                    # bf16 pairs per slab table

_cache = {}


def _host_geometry(spacing, sdr, rotations, translations):
    """Per-ray launch parameters, float32 (matches reference numerics)."""
    sp = np.asarray(spacing, F32)
    sdrf = F32(np.asarray(sdr).reshape(-1)[0])
    rot = np.asarray(rotations, F32)
    tr = np.asarray(translations, F32)
    th, ph, ga = rot[0]
    ct, st = np.cos(th, dtype=F32), np.sin(th, dtype=F32)
    cp, spn = np.cos(ph, dtype=F32), np.sin(ph, dtype=F32)
    cg, sg = np.cos(ga, dtype=F32), np.sin(ga, dtype=F32)
    Rz = np.array([[ct, -st, 0], [st, ct, 0], [0, 0, 1]], F32)
    Ry = np.array([[cp, 0, spn], [0, 1, 0], [-spn, 0, cp]], F32)
    Rx = np.array([[1, 0, 0], [0, cg, -sg], [0, sg, cg]], F32)
    Rm = (Rz @ Ry @ Rx).astype(F32)
    source3 = (sdrf * Rm[:, 0]).astype(F32)
    center3 = (-source3).astype(F32)
    basis = np.stack([Rm[:, 1], Rm[:, 2]]).astype(F32)
    source3 = source3 + tr[0]
    center3 = center3 + tr[0]
    t = (np.arange(-(H // 2), H // 2, dtype=F32) + F32(1.0)) * F32(DELX)
    s = (np.arange(-(W // 2), W // 2, dtype=F32) + F32(1.0)) * F32(DELY)
    coefs = np.stack(np.meshgrid(t, s, indexing="ij"), -1).reshape(-1, 2).astype(F32)
    target = (coefs @ basis + center3).astype(F32)
    sdd = (target - source3 + F32(EPS)).astype(F32)

    a0 = ((F32(0.0) - source3) / sdd).astype(F32)
    extent = (np.array([NXV, NYV, NZV], F32) * sp).astype(F32)
    a1 = ((extent - source3) / sdd).astype(F32)
    amin = np.minimum(a0, a1).max(-1).astype(F32)
    amax = np.maximum(a0, a1).min(-1).astype(F32)
    rnorm = np.sqrt((sdd * sdd).sum(-1)).astype(F32)

    hx, hy, hz = sp
    sx, sy, sz = source3
    assert np.all(sdd[:, 0] < 0), "kernel assumes sddx < 0 for all rays"
    Dy = np.where(sdd[:, 1] > 0, F32(1.0), F32(-1.0)).astype(F32)
    Dz = np.where(sdd[:, 2] > 0, F32(1.0), F32(-1.0)).astype(F32)
    rsy = (F32(1.0) / sdd[:, 1]).astype(F32)
    rsz = (F32(1.0) / sdd[:, 2]).astype(F32)

    p = np.zeros((NP2, NRAY), F32)
    p[0] = (F32(1.0) / sdd[:, 0]).astype(F32)   # rsx
    p[1] = amin
    p[2] = amax
    p[3] = sdd[:, 1] * Dy / hy                  # aly (>0)
    p[4] = sdd[:, 2] * Dz / hz                  # alz
    p[5] = sy * Dy / hy - F32(0.5)              # bey' (-0.5 folded for rne)
    p[6] = sz * Dz / hz - F32(0.5)              # bez'
    # event-slot-expanded quads [a,e]: Dh, rs, ss duplicated per event
    p[7] = p[8] = Dy * hy                       # Dh4
    p[9] = p[10] = Dz * hz
    p[11] = p[12] = rsy                         # rs4
    p[13] = p[14] = rsz
    p[15] = p[16] = sy * rsy                    # ss4
    p[17] = p[18] = sz * rsz
    p[19] = sdd[:, 1] / (F32(2.0) * hy)         # ky
    p[20] = sdd[:, 2] / (F32(2.0) * hz)         # kz
    p[21] = sy / hy - F32(0.5)                  # cy' (-0.5 folded)
    p[22] = sz / hz - F32(0.5)                  # cz'
    scal = dict(hx=F32(hx), sx=F32(sx))
    return p, rnorm, scal


def _build(scal, reps=1):
    hx, sx = float(scal["hx"]), float(scal["sx"])

    nc = bacc.Bacc("TRN2", target_bir_lowering=False, debug=False,
                   num_devices=1)
    vol_in = nc.dram_tensor("vol", [2, 128, NE * 2], BF16,
                            kind="ExternalInput")
    par_in = nc.dram_tensor("par", [NCH, 1, NP2 * RC], MF32,
                            kind="ExternalInput")
    rno_in = nc.dram_tensor("rno", [1, NPAD], MF32, kind="ExternalInput")
    msk_in = nc.dram_tensor("msk", [128, 32], BF16, kind="ExternalInput")
    drr_out = nc.dram_tensor("drr", [1, NPAD], MF32, kind="ExternalOutput")
    s_hbm = nc.dram_tensor("s_part", [2, 128, NPAD], MF32)

    R4 = 4 * RC

    with tile.TileContext(nc) as tc:
        with tc.tile_pool(name="pool", bufs=1) as pool:
            table = pool.tile([128, NE * 2], BF16, tag="table")
            msk = pool.tile([128, 32], BF16, tag="msk")
            nc.sync.dma_start(msk[:], msk_in[:])
            jci = pool.tile([128, 1], mybir.dt.int32, tag="jci")
            jc0 = pool.tile([128, 1], MF32, tag="jc0")
            nc.gpsimd.iota(jci[:], pattern=[[0, 1]], base=0,
                           channel_multiplier=1)
            nc.vector.tensor_copy(jc0[:], jci[:])
            njc = pool.tile([128, 2], MF32, tag="njc")

            STT = nc.vector.scalar_tensor_tensor
            TS = nc.vector.tensor_scalar

            for rep in range(reps):
              for g in range(2):
                nc.gpsimd.dma_start(table[:], vol_in[g])
                # njc0 = j*hx - sx; njc1 = njc0 + hx   (j = p + 128 g)
                TS(njc[:, 0:1], jc0[:], hx, 128.0 * g * hx - sx,
                   AOP.mult, AOP.add)
                TS(njc[:, 1:2], njc[:, 0:1], hx, None, AOP.add)
                for ch in range(NCH):
                    pt = pool.tile([128, NP2 * RC], MF32, tag="pt")
                    gt = pool.tile([128, R4 * 16 * 2], BF16, tag="gt")
                    bb = pool.tile([128, 5 * RC], MF32, tag="bb")
                    mm = pool.tile([128, R4], MF32, tag="mm")
                    g0 = pool.tile([128, 2 * RC], MF32, tag="g0")
                    wgt = pool.tile([128, R4], BF16, tag="wgt")
                    pos = pool.tile([128, R4 * 2], MF32, tag="pos")
                    idx16 = pool.tile([128, R4], mybir.dt.int16, tag="idx16")
                    Wt = pool.tile([128, R4 * 2], BF16, tag="Wt")
                    sch = pool.tile([128, RC], MF32, tag="sch")

                    P = lambda k: pt[:, k * RC:(k + 1) * RC]
                    PA = lambda k: pt[:, k * RC:(k + 2) * RC].rearrange(
                        "p (a r) -> p a r", a=2)
                    bb5 = bb[:].rearrange("p (s r) -> p s r", s=5)
                    mm4 = mm[:].rearrange("p (a e r) -> p a e r", a=2, e=2)
                    g02 = g0[:].rearrange("p (a r) -> p a r", a=2)
                    pos2 = pos[:].rearrange("p (a f) -> p a f", a=2)
                    iy = pos2[:, 0, :]
                    iz = pos2[:, 1, :]
                    odd = iz              # odd computed in place on iz
                    W3 = Wt[:].rearrange("p (i c) -> p i c", c=2)
                    hi = g0[:, 0:RC]      # g0 dead after mm; reuse as hi

                    nc.sync.dma_start(
                        pt[:], par_in[ch].broadcast_to((128, NP2 * RC)))
                    # ta/tb = njc * rsx  (sddx<0 so tb < ta)
                    tab = g02
                    STT(tab, njc[:].rearrange("p (a r) -> p a r",
                                              r=1).broadcast_to(
                        (128, 2, RC)), 1.0,
                        P(0).rearrange("p (a r) -> p a r", a=1).broadcast_to(
                            (128, 2, RC)), AOP.bypass, AOP.mult)
                    # t0 = max(tb, amin); t1 = min(ta, amax); t1 = max(t1,t0)
                    STT(bb5[:, 0, :], g02[:, 1, :], 1.0, P(1),
                        AOP.bypass, AOP.max)
                    STT(bb5[:, 4, :], g02[:, 0, :], 1.0, P(2),
                        AOP.bypass, AOP.min)
                    STT(bb5[:, 4, :], bb5[:, 4, :], 1.0, bb5[:, 0, :],
                        AOP.bypass, AOP.max)
                    # g0 = t0*al + be'  (both axes; -0.5 folded into be')
                    t0b = bb[:, 0:RC].rearrange(
                        "p (a r) -> p a r", a=1).broadcast_to((128, 2, RC))
                    STT(g02, t0b, 1.0, PA(3), AOP.bypass, AOP.mult)
                    STT(g02, g02, 1.0, PA(5), AOP.bypass, AOP.add)
                    # F0 = rne(g0) (= floor of unshifted g)
                    TS(g0[:], g0[:], float(MAGIC), float(MAGIC),
                       AOP.add, AOP.subtract)
                    # m~ = (F0 + {1,2}) * Dh ; e = m~ * rs - ss (in place)
                    # split per event slot: walrus rejects >2 free dims
                    F02 = g0[:].rearrange("p (a r) -> p a r", a=2)
                    TS(mm4[:, :, 0, :], F02, 1.0, None, AOP.add)
                    TS(mm4[:, :, 1, :], F02, 2.0, None, AOP.add)
                    # e = m~*Dh*rs - ss, dense quads (host-expanded)
                    STT(mm[:], mm[:], 1.0, pt[:, 7 * RC:11 * RC],
                        AOP.bypass, AOP.mult)
                    STT(mm[:], mm[:], 1.0, pt[:, 11 * RC:15 * RC],
                        AOP.bypass, AOP.mult)
                    STT(mm[:], mm[:], 1.0, pt[:, 15 * RC:19 * RC],
                        AOP.bypass, AOP.subtract)
                    # merge: 3 smallest of {e1y,e2y,e1z,e2z} -> x1,x2,x3
                    STT(bb5[:, 1:4:2, :], mm4[:, 0, :, :], 1.0,
                        mm4[:, 1, :, :], AOP.bypass, AOP.min)
                    STT(hi, mm[:, 0:RC], 1.0, mm[:, 2 * RC:3 * RC],
                        AOP.bypass, AOP.max)
                    STT(bb5[:, 2, :], hi, 1.0, bb5[:, 3, :],
                        AOP.bypass, AOP.min)
                    STT(bb5[:, 3, :], hi, 1.0, bb5[:, 3, :],
                        AOP.bypass, AOP.max)
                    # events are >= t0 by construction; only clip above t1
                    t1c = bb[:, 4 * RC:5 * RC].rearrange(
                        "p (a r) -> p a r", a=1).broadcast_to((128, 3, RC))
                    STT(bb5[:, 1:4, :], bb5[:, 1:4, :], 1.0, t1c,
                        AOP.bypass, AOP.min)
                    # widths and midpoint sums
                    STT(wgt[:], bb[:, RC:5 * RC], 1.0, bb[:, 0:4 * RC],
                        AOP.bypass, AOP.subtract)
                    STT(mm[:], bb[:, 0:4 * RC], 1.0, bb[:, RC:5 * RC],
                        AOP.bypass, AOP.add)   # mm = bL+bR (e-tile dead)
                    # pos = sum*k + c'  (per axis; k,c per-ray bcast over s)
                    for a in range(2):
                        pa = pos2[:, a, :].rearrange("p (s r) -> p s r", s=4)
                        kB = P(19 + a).rearrange(
                            "p (s r) -> p s r", s=1).broadcast_to(
                            (128, 4, RC))
                        cB = P(21 + a).rearrange(
                            "p (s r) -> p s r", s=1).broadcast_to(
                            (128, 4, RC))
                        STT(pa, mm[:].rearrange("p (s r) -> p s r", s=4),
                            1.0, kB, AOP.bypass, AOP.mult)
                        STT(pa, pa, 1.0, cB, AOP.bypass, AOP.add)
                    # iy/iz = clamp(rne(pos'), 0, 255)
                    TS(pos[:], pos[:], float(MAGIC), float(MAGIC),
                       AOP.add, AOP.subtract)
                    TS(pos[:], pos[:], 0.0, 255.0, AOP.max, AOP.min)
                    # izh = floor(iz/2) = rne(iz*0.5 - 0.25) (no ties);
                    # odd = iz - 2*izh (in place on iz); idx = iy*128 + izh
                    izh = mm[:]
                    TS(izh, iz, 0.5, -0.25, AOP.mult, AOP.add)
                    TS(izh, izh, float(MAGIC), float(MAGIC),
                       AOP.add, AOP.subtract)
                    STT(odd, izh, -2.0, iz, AOP.mult, AOP.add)
                    STT(idx16[:], iy, 128.0, izh, AOP.mult, AOP.add)
                    # interleaved pair weights: Whi = wgt*odd, Wlo = wgt-Whi
                    STT(W3[:, :, 1], wgt[:], 1.0, odd, AOP.bypass, AOP.mult)
                    STT(W3[:, :, 0], wgt[:], 1.0, W3[:, :, 1],
                        AOP.bypass, AOP.subtract)
                    nc.gpsimd.ap_gather(
                        out_ap=gt[:], in_ap=table[:], idxs_ap=idx16[:],
                        channels=128, num_elems=NE, d=2, num_idxs=16 * R4)
                    # select: mask lanes (q,c expanded msk32), sum q, apply
                    # interleaved weights, fused (slot, c) reduce
                    gt3 = gt[:].rearrange("p (i qc) -> p i qc", qc=32)
                    mskb = msk[:].rearrange(
                        "p (i qc) -> p i qc", i=1).broadcast_to(
                        (128, R4, 32))
                    STT(gt3, gt3, 1.0, mskb, AOP.bypass, AOP.mult)
                    # q-reduce scratch reuses the pos slot (pos is dead)
                    ured = pool.tile([128, R4 * 2], MF32, tag="pos",
                                     name="ured")
                    nc.vector.tensor_reduce(
                        ured[:],
                        gt[:].rearrange("p (i q c) -> p i c q", q=16, c=2),
                        axis=mybir.AxisListType.X, op=AOP.add)
                    STT(ured[:], ured[:], 1.0, Wt[:], AOP.bypass, AOP.mult)
                    nc.vector.tensor_reduce(
                        sch[:],
                        ured[:].rearrange("p (s r c) -> p r s c", s=4, c=2),
                        axis=mybir.AxisListType.XY, op=AOP.add)
                    nc.sync.dma_start(s_hbm[g, :, ch * RC:(ch + 1) * RC],
                                      sch[:])

        # DRR[r] = rnorm[r] * sum_p (s0[p,r] + s1[p,r])
        with tc.tile_pool(name="fin", bufs=1) as fin, \
             tc.tile_pool(name="ppsum", bufs=2, space="PSUM") as ppsum:
            s0 = fin.tile([128, NPAD], MF32, tag="s0")
            s1 = fin.tile([128, NPAD], MF32, tag="s1")
            nc.sync.dma_start(s0[:], s_hbm[0])
            nc.sync.dma_start(s1[:], s_hbm[1])
            nc.vector.scalar_tensor_tensor(s0[:], s0[:], 1.0, s1[:],
                                           AOP.bypass, AOP.add)
            ones = fin.tile([128, 1], MF32, tag="ones")
            nc.vector.memset(ones[:], 1.0)
            orow = fin.tile([1, NPAD], MF32, tag="orow")
            CH2 = 512
            nblk = (NPAD + CH2 - 1) // CH2
            for nb in range(nblk):
                lo = nb * CH2
                hic = min(NPAD, lo + CH2)
                ps = ppsum.tile([1, CH2], MF32)
                nc.tensor.matmul(ps[:, 0:hic - lo], ones[:], s0[:, lo:hic],
                                 start=True, stop=True)
                nc.vector.tensor_copy(orow[:, lo:hic], ps[:, 0:hic - lo])
            rnc = fin.tile([1, NPAD], MF32, tag="rnc")
            nc.sync.dma_start(rnc[:], rno_in[:])
            nc.vector.scalar_tensor_tensor(orow[:], orow[:], 1.0, rnc[:],
                                           AOP.bypass, AOP.mult)
            nc.sync.dma_start(drr_out[:], orow[:])

    nc.finalize()
    return nc


def _prep_inputs(volume, spacing, sdr, rotations, translations):
    vol = np.asarray(volume, F32)
    par, rnorm, scal = _host_geometry(spacing, sdr, rotations, translations)
    volf = vol[::-1].astype(bfloat16)
    vol_dev = np.ascontiguousarray(volf.reshape(2, 128, NE * 2))
    mk = np.repeat((np.arange(128)[:, None] % 16 ==
                    np.arange(16)[None, :]), 2, axis=1).astype(bfloat16)
    in_maps = []
    for c in range(NCORES):
        lo = c * NPC
        pc = np.zeros((NP2, NPAD), F32)
        pc[:, :NPC] = par[:, lo:lo + NPC]
        pc[:, NPC:] = pc[:, :1]
        pch = np.ascontiguousarray(
            pc.reshape(NP2, NCH, RC).transpose(1, 0, 2)).reshape(
            NCH, 1, NP2 * RC)
        rn = np.zeros((1, NPAD), F32)
        rn[0, :NPC] = rnorm[lo:lo + NPC]
        in_maps.append({"vol": vol_dev, "par": pch, "rno": rn, "msk": mk})
    return in_maps, scal


def kernel(volume, spacing, sdr, rotations, translations):
    in_maps, scal = _prep_inputs(volume, spacing, sdr, rotations,
                                 translations)
    if "nc" not in _cache:
        _cache["nc"] = _build(scal)
    nc = _cache["nc"]
    _cache["in_maps"] = in_maps
    res = run_bass_kernel_spmd(nc, in_maps, list(range(NCORES)))
    drr = np.concatenate([res.results[c]["drr"][0, :NPC]
                          for c in range(NCORES)])
    return drr.reshape(1, H, W).astype(F32)


# revision 10
# speedup vs baseline: 5.7285x; 1.0524x over previous
"""Siddon DRR kernel v2 for 8 Trainium2 NeuronCores.

Instruction-count-minimized redesign. Measured backend cost model:
~28us fixed per DVE op + ~3.5ns/elem, ap_gather ~45us + 34ns/idx,
DMA ~50us, no cross-engine overlap. So: few, wide ops.

Structure (per core): volume (flipped, bf16) as 256 x-slab tables in 2
generations of 128 partitions. Rays chunked RC=176 x 29 chunks. Per
(ray, slab) window [t0,t1]: at most 2 y- and 2 z-plane crossings; the 3
smallest events + window ends give 4 Siddon sub-intervals. Events from
floor(g(t0)) via round-to-nearest MAGIC with -0.5 host-folded offsets
(exact-integer edge cases accepted). Voxel fetch: one ap_gather of 4
slots x 16-shared-idx pairs; select via mask+interleaved-weight mults
and a fused (q,c)-axis reduce. Cross-partition sum via PE ones-matmul.
"""
import sys
sys.path.insert(0, "/opt/trn_rl_repo")
import numpy as np

import concourse.bass as bass
import concourse.bacc as bacc
import concourse.tile as tile
import concourse.mybir as mybir
from concourse.bass_utils import run_bass_kernel_spmd
from ml_dtypes import bfloat16

F32 = np.float32
AOP = mybir.AluOpType
BF16 = mybir.dt.bfloat16
MF32 = mybir.dt.float32

H = W = 200
NXV = NYV = NZV = 256
DELX = DELY = 1.5
EPS = 1e-8
NCORES = 8
NRAY = H * W
NPC = NRAY // NCORES          # rays per core (5000)
RC = 176                      # rays per chunk
NCH = 29                      # chunks per generation
NPAD = RC * NCH               # padded rays per core (5104)
NP2 = 23                      # packed params per ray
MAGIC = F32(12582912.0)       # 2^23 + 2^22: rne for |x|<2^22
NE = 32768                    # bf16 pairs per slab table

_cache = {}


def _host_geometry(spacing, sdr, rotations, translations):
    """Per-ray launch parameters, float32 (matches reference numerics)."""
    sp = np.asarray(spacing, F32)
    sdrf = F32(np.asarray(sdr).reshape(-1)[0])
    rot = np.asarray(rotations, F32)
    tr = np.asarray(translations, F32)
    th, ph, ga = rot[0]
    ct, st = np.cos(th, dtype=F32), np.sin(th, dtype=F32)
    cp, spn = np.cos(ph, dtype=F32), np.sin(ph, dtype=F32)
    cg, sg = np.cos(ga, dtype=F32), np.sin(ga, dtype=F32)
    Rz = np.array([[ct, -st, 0], [st, ct, 0], [0, 0, 1]], F32)
    Ry = np.array([[cp, 0, spn], [0, 1, 0], [-spn, 0, cp]], F32)
    Rx = np.array([[1, 0, 0], [0, cg, -sg], [0, sg, cg]], F32)
    Rm = (Rz @ Ry @ Rx).astype(F32)
    source3 = (sdrf * Rm[:, 0]).astype(F32)
    center3 = (-source3).astype(F32)
    basis = np.stack([Rm[:, 1], Rm[:, 2]]).astype(F32)
    source3 = source3 + tr[0]
    center3 = center3 + tr[0]
    t = (np.arange(-(H // 2), H // 2, dtype=F32) + F32(1.0)) * F32(DELX)
    s = (np.arange(-(W // 2), W // 2, dtype=F32) + F32(1.0)) * F32(DELY)
    coefs = np.stack(np.meshgrid(t, s, indexing="ij"), -1).reshape(-1, 2).astype(F32)
    target = (coefs @ basis + center3).astype(F32)
    sdd = (target - source3 + F32(EPS)).astype(F32)

    a0 = ((F32(0.0) - source3) / sdd).astype(F32)
    extent = (np.array([NXV, NYV, NZV], F32) * sp).astype(F32)
    a1 = ((extent - source3) / sdd).astype(F32)
    amin = np.minimum(a0, a1).max(-1).astype(F32)
    amax = np.maximum(a0, a1).min(-1).astype(F32)
    rnorm = np.sqrt((sdd * sdd).sum(-1)).astype(F32)

    hx, hy, hz = sp
    sx, sy, sz = source3
    assert np.all(sdd[:, 0] < 0), "kernel assumes sddx < 0 for all rays"
    Dy = np.where(sdd[:, 1] > 0, F32(1.0), F32(-1.0)).astype(F32)
    Dz = np.where(sdd[:, 2] > 0, F32(1.0), F32(-1.0)).astype(F32)
    rsy = (F32(1.0) / sdd[:, 1]).astype(F32)
    rsz = (F32(1.0) / sdd[:, 2]).astype(F32)

    p = np.zeros((NP2, NRAY), F32)
    p[0] = (F32(1.0) / sdd[:, 0]).astype(F32)   # rsx
    p[1] = amin
    p[2] = amax
    p[3] = sdd[:, 1] * Dy / hy                  # aly (>0)
    p[4] = sdd[:, 2] * Dz / hz                  # alz
    p[5] = sy * Dy / hy - F32(0.5)              # bey' (-0.5 folded for rne)
    p[6] = sz * Dz / hz - F32(0.5)              # bez'
    # event-slot-expanded quads [a,e]: Dh, rs, ss duplicated per event
    p[7] = p[8] = Dy * hy                       # Dh4
    p[9] = p[10] = Dz * hz
    p[11] = p[12] = rsy                         # rs4
    p[13] = p[14] = rsz
    p[15] = p[16] = sy * rsy                    # ss4
    p[17] = p[18] = sz * rsz
    p[19] = sdd[:, 1] / (F32(2.0) * hy)         # ky
    p[20] = sdd[:, 2] / (F32(2.0) * hz)         # kz
    p[21] = sy / hy - F32(0.5)                  # cy' (-0.5 folded)
    p[22] = sz / hz - F32(0.5)                  # cz'
    scal = dict(hx=F32(hx), sx=F32(sx))
    return p, rnorm, scal


def _build(scal, reps=1):
    hx, sx = float(scal["hx"]), float(scal["sx"])

    nc = bacc.Bacc("TRN2", target_bir_lowering=False, debug=False,
                   num_devices=1)
    vol_in = nc.dram_tensor("vol", [2, 128, NE * 2], BF16,
                            kind="ExternalInput")
    par_in = nc.dram_tensor("par", [NCH, 1, NP2 * RC], MF32,
                            kind="ExternalInput")
    rno_in = nc.dram_tensor("rno", [1, NPAD], MF32, kind="ExternalInput")
    msk_in = nc.dram_tensor("msk", [128, 32], BF16, kind="ExternalInput")
    drr_out = nc.dram_tensor("drr", [1, NPAD], MF32, kind="ExternalOutput")
    s_hbm = nc.dram_tensor("s_part", [2, 128, NPAD], MF32)

    R4 = 4 * RC

    with tile.TileContext(nc) as tc:
        with tc.tile_pool(name="pool", bufs=1) as pool:
            table = pool.tile([128, NE * 2], BF16, tag="table")
            msk = pool.tile([128, 32], BF16, tag="msk")
            nc.sync.dma_start(msk[:], msk_in[:])
            jci = pool.tile([128, 1], mybir.dt.int32, tag="jci")
            jc0 = pool.tile([128, 1], MF32, tag="jc0")
            nc.gpsimd.iota(jci[:], pattern=[[0, 1]], base=0,
                           channel_multiplier=1)
            nc.vector.tensor_copy(jc0[:], jci[:])
            njc = pool.tile([128, 2], MF32, tag="njc")

            STT = nc.vector.scalar_tensor_tensor
            TS = nc.vector.tensor_scalar

            for rep in range(reps):
              for g in range(2):
                nc.gpsimd.dma_start(table[:], vol_in[g])
                # njc0 = j*hx - sx; njc1 = njc0 + hx   (j = p + 128 g)
                TS(njc[:, 0:1], jc0[:], hx, 128.0 * g * hx - sx,
                   AOP.mult, AOP.add)
                TS(njc[:, 1:2], njc[:, 0:1], hx, None, AOP.add)
                for ch in range(NCH):
                    pt = pool.tile([128, NP2 * RC], MF32, tag="pt")
                    gt = pool.tile([128, R4 * 16 * 2], BF16, tag="gt")
                    bb = pool.tile([128, 5 * RC], MF32, tag="bb")
                    mm = pool.tile([128, R4], MF32, tag="mm")
                    g0 = pool.tile([128, 2 * RC], MF32, tag="g0")
                    wgt = pool.tile([128, R4], BF16, tag="wgt")
                    pos = pool.tile([128, R4 * 2], MF32, tag="pos")
                    idx16 = pool.tile([128, R4], mybir.dt.int16, tag="idx16")
                    Wt = pool.tile([128, R4 * 2], BF16, tag="Wt")
                    sch = pool.tile([128, RC], MF32, tag="sch")

                    P = lambda k: pt[:, k * RC:(k + 1) * RC]
                    PA = lambda k: pt[:, k * RC:(k + 2) * RC].rearrange(
                        "p (a r) -> p a r", a=2)
                    bb5 = bb[:].rearrange("p (s r) -> p s r", s=5)
                    mm4 = mm[:].rearrange("p (a e r) -> p a e r", a=2, e=2)
                    g02 = g0[:].rearrange("p (a r) -> p a r", a=2)
                    pos2 = pos[:].rearrange("p (a f) -> p a f", a=2)
                    iy = pos2[:, 0, :]
                    iz = pos2[:, 1, :]
                    odd = iz              # odd computed in place on iz
                    W3 = Wt[:].rearrange("p (i c) -> p i c", c=2)
                    hi = g0[:, 0:RC]      # g0 dead after mm; reuse as hi

                    nc.sync.dma_start(
                        pt[:], par_in[ch].broadcast_to((128, NP2 * RC)))
                    # ta/tb = njc * rsx  (sddx<0 so tb < ta)
                    tab = g02
                    STT(tab, njc[:].rearrange("p (a r) -> p a r",
                                              r=1).broadcast_to(
                        (128, 2, RC)), 1.0,
                        P(0).rearrange("p (a r) -> p a r", a=1).broadcast_to(
                            (128, 2, RC)), AOP.bypass, AOP.mult)
                    # t0 = max(tb, amin); t1 = min(ta, amax); t1 = max(t1,t0)
                    STT(bb5[:, 0, :], g02[:, 1, :], 1.0, P(1),
                        AOP.bypass, AOP.max)
                    STT(bb5[:, 4, :], g02[:, 0, :], 1.0, P(2),
                        AOP.bypass, AOP.min)
                    STT(bb5[:, 4, :], bb5[:, 4, :], 1.0, bb5[:, 0, :],
                        AOP.bypass, AOP.max)
                    # g0 = t0*al + be'  (both axes; -0.5 folded into be')
                    t0b = bb[:, 0:RC].rearrange(
                        "p (a r) -> p a r", a=1).broadcast_to((128, 2, RC))
                    STT(g02, t0b, 1.0, PA(3), AOP.bypass, AOP.mult)
                    STT(g02, g02, 1.0, PA(5), AOP.bypass, AOP.add)
                    # F0 = rne(g0) (= floor of unshifted g)
                    TS(g0[:], g0[:], float(MAGIC), float(MAGIC),
                       AOP.add, AOP.subtract)
                    # m~ = (F0 + {1,2}) * Dh ; e = m~ * rs - ss (in place)
                    # split per event slot: walrus rejects >2 free dims
                    F02 = g0[:].rearrange("p (a r) -> p a r", a=2)
                    TS(mm4[:, :, 0, :], F02, 1.0, None, AOP.add)
                    TS(mm4[:, :, 1, :], F02, 2.0, None, AOP.add)
                    # e = m~*Dh*rs - ss, dense quads (host-expanded)
                    STT(mm[:], mm[:], 1.0, pt[:, 7 * RC:11 * RC],
                        AOP.bypass, AOP.mult)
                    STT(mm[:], mm[:], 1.0, pt[:, 11 * RC:15 * RC],
                        AOP.bypass, AOP.mult)
                    STT(mm[:], mm[:], 1.0, pt[:, 15 * RC:19 * RC],
                        AOP.bypass, AOP.subtract)
                    # merge: 3 smallest of {e1y,e2y,e1z,e2z} -> x1,x2,x3
                    STT(bb5[:, 1:4:2, :], mm4[:, 0, :, :], 1.0,
                        mm4[:, 1, :, :], AOP.bypass, AOP.min)
                    STT(hi, mm[:, 0:RC], 1.0, mm[:, 2 * RC:3 * RC],
                        AOP.bypass, AOP.max)
                    STT(bb5[:, 2, :], hi, 1.0, bb5[:, 3, :],
                        AOP.bypass, AOP.min)
                    STT(bb5[:, 3, :], hi, 1.0, bb5[:, 3, :],
                        AOP.bypass, AOP.max)
                    # events are >= t0 by construction; only clip above t1
                    t1c = bb[:, 4 * RC:5 * RC].rearrange(
                        "p (a r) -> p a r", a=1).broadcast_to((128, 3, RC))
                    STT(bb5[:, 1:4, :], bb5[:, 1:4, :], 1.0, t1c,
                        AOP.bypass, AOP.min)
                    # widths and midpoint sums
                    STT(wgt[:], bb[:, RC:5 * RC], 1.0, bb[:, 0:4 * RC],
                        AOP.bypass, AOP.subtract)
                    STT(mm[:], bb[:, 0:4 * RC], 1.0, bb[:, RC:5 * RC],
                        AOP.bypass, AOP.add)   # mm = bL+bR (e-tile dead)
                    # pos = sum*k + c'  (per axis; k,c per-ray bcast over s)
                    for a in range(2):
                        pa = pos2[:, a, :].rearrange("p (s r) -> p s r", s=4)
                        kB = P(19 + a).rearrange(
                            "p (s r) -> p s r", s=1).broadcast_to(
                            (128, 4, RC))
                        cB = P(21 + a).rearrange(
                            "p (s r) -> p s r", s=1).broadcast_to(
                            (128, 4, RC))
                        STT(pa, mm[:].rearrange("p (s r) -> p s r", s=4),
                            1.0, kB, AOP.bypass, AOP.mult)
                        STT(pa, pa, 1.0, cB, AOP.bypass, AOP.add)
                    # iy/iz = clamp(rne(pos'), 0, 255)
                    TS(pos[:], pos[:], float(MAGIC), float(MAGIC),
                       AOP.add, AOP.subtract)
                    TS(pos[:], pos[:], 0.0, 255.0, AOP.max, AOP.min)
                    # izh = floor(iz/2) = rne(iz*0.5 - 0.25) (no ties);
                    # odd = iz - 2*izh (in place on iz); idx = iy*128 + izh
                    izh = mm[:]
                    TS(izh, iz, 0.5, -0.25, AOP.mult, AOP.add)
                    TS(izh, izh, float(MAGIC), float(MAGIC),
                       AOP.add, AOP.subtract)
                    STT(odd, izh, -2.0, iz, AOP.mult, AOP.add)
                    STT(idx16[:], iy, 128.0, izh, AOP.mult, AOP.add)
                    # interleaved pair weights: Whi = wgt*odd, Wlo = wgt-Whi
                    STT(W3[:, :, 1], wgt[:], 1.0, odd, AOP.bypass, AOP.mult)
                    STT(W3[:, :, 0], wgt[:], 1.0, W3[:, :, 1],
                        AOP.bypass, AOP.subtract)
                    nc.gpsimd.ap_gather(
                        out_ap=gt[:], in_ap=table[:], idxs_ap=idx16[:],
                        channels=128, num_elems=NE, d=2, num_idxs=16 * R4)
                    # select: mask lanes (q,c expanded msk32), sum q, apply
                    # interleaved weights, fused (slot, c) reduce
                    gt3 = gt[:].rearrange("p (i qc) -> p i qc", qc=32)
                    mskb = msk[:].rearrange(
                        "p (i qc) -> p i qc", i=1).broadcast_to(
                        (128, R4, 32))
                    STT(gt3, gt3, 1.0, mskb, AOP.bypass, AOP.mult)
                    # q-reduce scratch reuses the pos slot (pos is dead)
                    ured = pool.tile([128, R4 * 2], MF32, tag="pos",
                                     name="ured")
                    nc.vector.tensor_reduce(
                        ured[:],
                        gt[:].rearrange("p (i q c) -> p i c q", q=16, c=2),
                        axis=mybir.AxisListType.X, op=AOP.add)
                    STT(ured[:], ured[:], 1.0, Wt[:], AOP.bypass, AOP.mult)
                    nc.vector.tensor_reduce(
                        sch[:],
                        ured[:].rearrange("p (s r c) -> p r s c", s=4, c=2),
                        axis=mybir.AxisListType.XY, op=AOP.add)
                    nc.sync.dma_start(s_hbm[g, :, ch * RC:(ch + 1) * RC],
                                      sch[:])

        # DRR[r] = rnorm[r] * sum_p (s0[p,r] + s1[p,r])
        with tc.tile_pool(name="fin", bufs=1) as fin, \
             tc.tile_pool(name="ppsum", bufs=2, space="PSUM") as ppsum:
            s0 = fin.tile([128, NPAD], MF32, tag="s0")
            s1 = fin.tile([128, NPAD], MF32, tag="s1")
            nc.sync.dma_start(s0[:], s_hbm[0])
            nc.sync.dma_start(s1[:], s_hbm[1])
            nc.vector.scalar_tensor_tensor(s0[:], s0[:], 1.0, s1[:],
                                           AOP.bypass, AOP.add)
            ones = fin.tile([128, 1], MF32, tag="ones")
            nc.vector.memset(ones[:], 1.0)
            orow = fin.tile([1, NPAD], MF32, tag="orow")
            CH2 = 512
            nblk = (NPAD + CH2 - 1) // CH2
            for nb in range(nblk):
                lo = nb * CH2
                hic = min(NPAD, lo + CH2)
                ps = ppsum.tile([1, CH2], MF32)
                nc.tensor.matmul(ps[:, 0:hic - lo], ones[:], s0[:, lo:hic],
                                 start=True, stop=True)
                nc.vector.tensor_copy(orow[:, lo:hic], ps[:, 0:hic - lo])
            rnc = fin.tile([1, NPAD], MF32, tag="rnc")
            nc.sync.dma_start(rnc[:], rno_in[:])
            nc.vector.scalar_tensor_tensor(orow[:], orow[:], 1.0, rnc[:],
                                           AOP.bypass, AOP.mult)
            nc.sync.dma_start(drr_out[:], orow[:])

    nc.finalize()
    return nc


def _prep_inputs(volume, spacing, sdr, rotations, translations):
    vol = np.asarray(volume, F32)
    par, rnorm, scal = _host_geometry(spacing, sdr, rotations, translations)
    volf = vol[::-1].astype(bfloat16)
    vol_dev = np.ascontiguousarray(volf.reshape(2, 128, NE * 2))
    mk = np.repeat((np.arange(128)[:, None] % 16 ==
                    np.arange(16)[None, :]), 2, axis=1).astype(bfloat16)
    in_maps = []
    for c in range(NCORES):
        lo = c * NPC
        pc = np.zeros((NP2, NPAD), F32)
        pc[:, :NPC] = par[:, lo:lo + NPC]
        pc[:, NPC:] = pc[:, :1]
        pch = np.ascontiguousarray(
            pc.reshape(NP2, NCH, RC).transpose(1, 0, 2)).reshape(
            NCH, 1, NP2 * RC)
        rn = np.zeros((1, NPAD), F32)
        rn[0, :NPC] = rnorm[lo:lo + NPC]
        in_maps.append({"vol": vol_dev, "par": pch, "rno": rn, "msk": mk})
    return in_maps, scal


def kernel(volume, spacing, sdr, rotations, translations):
    in_maps, scal = _prep_inputs(volume, spacing, sdr, rotations,
                                 translations)
    key = ("nc", float(scal["hx"]), float(scal["sx"]))
    if key not in _cache:
        _cache[key] = _build(scal)
    nc = _cache[key]
    _cache["in_maps"] = in_maps
    res = run_bass_kernel_spmd(nc, in_maps, list(range(NCORES)))
    drr = np.concatenate([res.results[c]["drr"][0, :NPC]
                          for c in range(NCORES)])
    return drr.reshape(1, H, W).astype(F32)
